# revision 66
# baseline (speedup 1.0000x reference)
"""Bass/Trainium2 SPMD kernel for a 2-layer GCN encoder.

Math (per reference):
    src/dst = edges + self-loops
    deg[v]  = #edges with dst==v (incl self-loop);  dinv = 1/sqrt(deg)
    layer(x, W, b): out[d] = dinv[d] * sum_{e: dst_e==d} dinv[src_e] * (x@W)[src_e] + b
    y = layer1(sigmoid(layer0(x, W0, b0)), W1, b1)

Distribution: nodes are sharded contiguously across 8 cores (6250 each).
Edges are owned by the destination core.  Each core:
  1. GEMM on its x rows, pre-scales rows by dinv (so the per-edge weight
     dinv[src]*dinv[dst] factorizes into a row pre-scale and an output
     post-scale), AllGathers the scaled features.
  2. For each 128-row destination block, gathers the source rows of its
     edges (dma_gather, int16 indices => the node table is split in two
     halves), builds one-hot scatter matrices on the vector engine
     (iota == slot), and scatter-adds via TensorE matmuls accumulating in
     PSUM.  Bias enters as a rank-1 matmul (sqrt(deg) x b), so the final
     PSUM->SBUF copy can apply the dinv post-scale (and sigmoid) in one
     ScalarE activation.
"""

import itertools
import math

import ml_dtypes
import numpy as np

import concourse.bacc as bacc
import concourse.bass as bass
import concourse.mybir as mybir
import concourse.tile as tile
from concourse.bass_utils import run_bass_kernel_spmd

P = 128
F32 = mybir.dt.float32
BF16 = mybir.dt.bfloat16
I16 = mybir.dt.int16

# Full-problem constants
N_NODES = 50000
N_CORES = 8
F0, F1, F2 = 128, 128, 64
GROUP_BLOCKS = 2  # dst blocks per dma_gather batch
# Per-(block,half) edge-segment alignment. Must stay 128: sub-128 matmul
# pieces with different base partitions back-to-back hard-crash the PE
# (verified on HW: K64@p0 directly followed by K64@p64 aborts the NEFF).
SEG_ALIGN = 128


def _round_up(x, m):
    return (x + m - 1) // m * m


class Plan:
    """Compile-time schedule, identical across cores (SPMD)."""

    def __init__(self, n_nodes, n_cores, gb):
        assert n_nodes % n_cores == 0
        self.n_nodes = n_nodes
        self.n_cores = n_cores
        self.npc = n_nodes // n_cores
        self.nblk = math.ceil(self.npc / P)
        # The feature table is AllGathered in two per-core row ranges
        # ([0,S1) and [S1,S1+S2) local rows, S2 padded to full blocks) so the
        # first half-table is available early and both halves stay under the
        # int16 gather-index limit.  S2 is made as large as int16 allows so
        # the first AllGather (after S1 GEMM blocks) fires as early as
        # possible.
        self.S2 = 32768 // n_cores // P * P
        self.S1 = self.nblk * P - self.S2
        assert 0 < self.S1 and n_cores * self.S1 <= 32768
        self.gb = gb
        self.groups = [
            list(range(i, min(i + gb, self.nblk))) for i in range(0, self.nblk, gb)
        ]
        self.g_of = {}
        for gi, blocks in enumerate(self.groups):
            for b in blocks:
                self.g_of[b] = gi
        # filled by finalize(): per-(blk, half) uniform padded sizes
        self.SZ = None  # [nblk, 2] int, multiples of SEG_ALIGN
        self.seg_off = {}  # (b, h) -> edge offset within its gather
        self.seg_idx16 = {}  # (g_idx, h) -> int16-column base of that gather
        self.seg_colbase = {}  # (g_idx, h) -> global chunk-column base
        self.gather_nid = {}  # (g_idx, h) -> num idxs
        self.ncols = 0
        self.tot16 = 0

    def finalize(self, sz):
        self.SZ = sz
        col = 0
        for gi, blocks in enumerate(self.groups):
            for h in (0, 1):
                off = 0
                for b in blocks:
                    self.seg_off[(b, h)] = off
                    off += int(self.SZ[b, h])
                self.gather_nid[(gi, h)] = off
                self.seg_colbase[(gi, h)] = col
                col += (off + P - 1) // P
        self.nid_max = _round_up(max(self.gather_nid.values()), P)
        i16 = 0
        for gi in range(len(self.groups)):
            for h in (0, 1):
                self.seg_idx16[(gi, h)] = i16
                i16 += self.gather_nid[(gi, h)] // 16
        self.ncols = col
        self.tot16 = i16


def _build_metadata(edges, n_nodes, n_cores, gb=GROUP_BLOCKS):
    """Host-side integer preprocessing: shard + sort edges, build gather
    indices / slot vectors / degree tables.  Returns (plan, per_core dict)."""
    plan = Plan(n_nodes, n_cores, gb)
    npc, nblk = plan.npc, plan.nblk
    S1, S2 = plan.S1, plan.S2

    loop = np.arange(n_nodes, dtype=np.int64)
    src = np.concatenate([np.asarray(edges[0], dtype=np.int64), loop])
    dst = np.concatenate([np.asarray(edges[1], dtype=np.int64), loop])
    deg = np.bincount(dst, minlength=n_nodes).astype(np.float32)

    owner = dst // npc
    ldst = dst % npc
    blk = ldst // P
    slot = (ldst % P).astype(np.float32)
    # gather-table position of each source node: half 0 = local rows [0,S1)
    # of every core (table1), half 1 = local rows [S1,S1+S2) (table2)
    sown = src // npc
    srow = src % npc
    half = (srow >= S1).astype(np.int64)
    tpos = np.where(half == 0, sown * S1 + srow, sown * S2 + (srow - S1))
    cell = ((owner * nblk) + blk) * 2 + half
    order = np.lexsort((src, cell))
    cell_s = cell[order]
    tpos_s = tpos[order]
    slot_s = slot[order]

    ncells = n_cores * nblk * 2
    counts = np.bincount(cell_s, minlength=ncells).reshape(n_cores, nblk, 2)
    starts = np.concatenate([[0], np.cumsum(counts.reshape(-1))])[:-1].reshape(
        n_cores, nblk, 2
    )
    sz = np.maximum(counts.max(axis=0), 0)
    sz = (np.ceil(sz / SEG_ALIGN).astype(np.int64)) * SEG_ALIGN  # [nblk, 2]
    plan.finalize(sz)

    ncols = plan.ncols
    tot16 = plan.tot16

    per_core = []
    for c in range(n_cores):
        idx16 = np.zeros((16, tot16), np.int16)
        slots_t = np.full((P, ncols), -1.0, np.float32)
        for gi, blocks in enumerate(plan.groups):
            for h in (0, 1):
                i16b = plan.seg_idx16[(gi, h)] * 16
                colb = plan.seg_colbase[(gi, h)] * P
                for b in blocks:
                    n = int(counts[c, b, h])
                    s0 = int(starts[c, b, h])
                    if n:
                        j = plan.seg_off[(b, h)] + np.arange(n)
                        seg_src = tpos_s[s0 : s0 + n].astype(np.int16)
                        ji = i16b + j
                        idx16[ji % 16, ji // 16] = seg_src
                        jc = colb + j
                        slots_t[jc % P, jc // P] = slot_s[s0 : s0 + n]
        deg_loc = np.ones(nblk * P, np.float32)
        deg_loc[:npc] = deg[c * npc : (c + 1) * npc]
        deg_t = deg_loc.reshape(nblk, P).T.copy()  # [P, nblk]
        per_core.append(
            dict(
                idx16=np.tile(idx16, (8, 1)),  # [128, tot16]
                slots=slots_t,
                degt=deg_t,
                degrow=deg_loc.reshape(1, -1).copy(),
            )
        )
    return plan, per_core


def _build_nc(plan, f0, f1, f2):
    """Build the SPMD bass program (same for every core)."""
    npc, nblk = plan.npc, plan.nblk
    S1, S2 = plan.S1, plan.S2
    ncores = plan.n_cores
    nsplit = S1 // P  # GEMM blocks staged into table 1
    nc = bacc.Bacc(
        "TRN2",
        target_bir_lowering=False,
        debug=False,
        num_devices=plan.n_cores,
        num_swdge_queues=4,
    )

    # I/O
    xT_d = nc.dram_tensor("xT", [f0, npc], F32, kind="ExternalInput")
    w0_d = nc.dram_tensor("W0", [f0, f1], F32, kind="ExternalInput")
    w1_d = nc.dram_tensor("W1", [f1, f2], F32, kind="ExternalInput")
    b0_d = nc.dram_tensor("b0", [1, f1], F32, kind="ExternalInput")
    b1_d = nc.dram_tensor("b1", [1, f2], F32, kind="ExternalInput")
    iota_d = nc.dram_tensor("iota", [P, P], BF16, kind="ExternalInput")
    iotar_d = nc.dram_tensor("iotar", [P, 16 * P], BF16, kind="ExternalInput")
    ident_d = nc.dram_tensor("ident", [P, P], F32, kind="ExternalInput")
    degt_d = nc.dram_tensor("degt", [P, nblk], F32, kind="ExternalInput")
    degrow_d = nc.dram_tensor("degrow", [1, nblk * P], F32, kind="ExternalInput")
    idx_d = nc.dram_tensor("idx16", [P, plan.tot16], I16, kind="ExternalInput")
    widx_d = nc.dram_tensor("widx", [P, 8], I16, kind="ExternalInput")
    slots_d = nc.dram_tensor("slots", [P, plan.ncols], BF16, kind="ExternalInput")
    y_d = nc.dram_tensor("y", [npc, f2], F32, kind="ExternalOutput")

    rg = [list(range(plan.n_cores))]
    AF = mybir.ActivationFunctionType

    with tile.TileContext(nc) as tc:
        with (
            tc.tile_pool(name="dram", bufs=1, space="DRAM") as dramp,
            tc.tile_pool(name="const", bufs=1) as constp,
            tc.tile_pool(name="gath", bufs=11) as gpool,
            tc.tile_pool(name="sel", bufs=5) as spool,
            tc.tile_pool(name="stage", bufs=4) as stpool,
            tc.tile_pool(name="x1p", bufs=4) as x1pool,
            tc.tile_pool(name="pgemm", bufs=2, space="PSUM") as pgemm,
            tc.tile_pool(name="pscat", bufs=3, space="PSUM") as pscat,
            tc.tile_pool(name="ptrans", bufs=1, space="PSUM") as ptrans,
        ):
            # Per-layer feature tables, AllGathered in two row ranges so the
            # first collective (and the half-0 gathers) fire early.  L1
            # tables are bf16 padded to 128 cols (gather elem must be a
            # multiple of 256B); cols f2:128 are never-read garbage.
            h1_locA = dramp.tile([S1, f1], BF16, name="h1_locA")
            h1_locB = dramp.tile([S2, f1], BF16, name="h1_locB")
            h1_t1 = dramp.tile(
                [ncores * S1, f1], BF16, addr_space="Shared", name="h1_t1"
            )
            h1_t2 = dramp.tile(
                [ncores * S2, f1], BF16, addr_space="Shared", name="h1_t2"
            )
            h2_locA = dramp.tile([S1, P], BF16, name="h2_locA")
            h2_locB = dramp.tile([S2, P], BF16, name="h2_locB")
            h2_t1 = dramp.tile(
                [ncores * S1, P], BF16, addr_space="Shared", name="h2_t1"
            )
            h2_t2 = dramp.tile(
                [ncores * S2, P], BF16, addr_space="Shared", name="h2_t2"
            )

            # ---- constants / metadata ----
            def load_const(name, dram, shape, dtype=F32):
                t = constp.tile(shape, dtype, name=name)
                nc.sync.dma_start(out=t[:], in_=dram[:])
                return t

            # ordered so the warm gathers + L0 GEMM -> AllGather chain start
            # ASAP; the big gather metadata loads overlap with it
            widx_t = load_const("widx_t", widx_d, [P, 8], I16)
            ident_t = load_const("ident_t", ident_d, [P, P])
            xT_t = load_const("xT_t", xT_d, [f0, npc])
            w0_t = load_const("w0_t", w0_d, [f0, f1])
            degt_t = load_const("degt_t", degt_d, [P, nblk])
            w1_t = load_const("w1_t", w1_d, [f1, f2])
            b0_t = load_const("b0_t", b0_d, [1, f1])
            b1_t = load_const("b1_t", b1_d, [1, f2])
            iota_t = load_const("iota_t", iota_d, [P, P], BF16)
            iotar_t = load_const("iotar_t", iotar_d, [P, 16 * P], BF16)
            degrow_t = load_const("degrow_t", degrow_d, [1, nblk * P])
            idx_t = load_const("idx_t", idx_d, [P, plan.tot16], I16)
            slots_t = load_const("slots_t", slots_d, [P, plan.ncols], BF16)

            # dinv = 1/sqrt(deg); sqdeg rows (flat, partition 0) for bias matmuls
            sq_t = constp.tile([P, nblk], F32, name="sq_t")
            nc.scalar.activation(sq_t[:], degt_t[:], AF.Sqrt)
            dinv_t = constp.tile([P, nblk], F32, name="dinv_t")
            nc.vector.reciprocal(dinv_t[:], sq_t[:])
            sqrow_t = constp.tile([1, nblk * P], F32, name="sqrow_t")
            nc.scalar.activation(sqrow_t[:], degrow_t[:], AF.Sqrt)

            # warm the Q7 dma_gather ucode on every SWDGE queue pair (each
            # pair pays its own ~29us icache fill; do it under the
            # GEMM+AllGather head instead)
            for q in range(4):
                warm_t = constp.tile([P, 1, 64], F32, name=f"warm_t{q}")
                nc.gpsimd.dma_gather(
                    warm_t[:],
                    ident_d[:, 0:64],
                    widx_t[:, 0:8],
                    128,
                    128,
                    64,
                    elem_step=P,
                    single_packet=False,
                    queue_num=q,
                )

            # SBUF staging for the local h rows of each layer: blocks land
            # here from the GEMMs, then two bulk DMAs + two AllGathers per
            # layer publish them as the gather tables.
            h1sb = constp.tile([P, nblk, f1], BF16, name="h1sb")
            h2sb = constp.tile([P, nblk, f2], BF16, name="h2sb")

            def publish(hsb, fstage, loc, tab, b_lo, b_hi):
                """DMA staged blocks [b_lo,b_hi) to local DRAM + AllGather.
                The DMA goes on the Scalar HWDGE queue so it does not queue
                behind the bulk constant loads on Sync."""
                nc.scalar.dma_start(
                    out=loc[:, 0:fstage].rearrange("(c p) f -> p c f", p=P),
                    in_=hsb[:, b_lo:b_hi, :],
                )
                nc.gpsimd.collective_compute(
                    "AllGather",
                    mybir.AluOpType.bypass,
                    replica_groups=rg,
                    ins=[loc[:, :].opt()],
                    outs=[tab[:, :].opt()],
                )

            def gemm_layer0():
                """h1sb[:, t] = dinv * (x @ W0); publish each half-table."""
                for t in range(nblk):
                    wt = min(P, npc - t * P)
                    hp = pgemm.tile([P, f1], F32, name="hp")
                    nc.tensor.matmul(
                        hp[:wt, :],
                        xT_t[:, t * P : t * P + wt],
                        w0_t[:],
                        start=True,
                        stop=True,
                    )
                    nc.scalar.activation(
                        h1sb[:wt, t, :],
                        hp[:wt, :],
                        AF.Copy,
                        scale=dinv_t[:wt, t : t + 1],
                    )
                    if t == nsplit - 1:
                        publish(h1sb, f1, h1_locA, h1_t1, 0, nsplit)
                publish(h1sb, f1, h1_locB, h1_t2, nsplit, nblk)

            qctr = itertools.count()

            def scatter_layer(
                t1, t2, fout, bias_t, is_last, hdt, felem=None,
                post_group=None, post_issue=None,
            ):
                """For every dst block: gather + one-hot matmul scatter-add.

                felem: gathered row width (table columns); the matmul only
                consumes the first `fout` of them.  Gathers rotate across the
                4 SWDGE queues so descriptor generation runs on all 4 Q7 core
                pairs concurrently.  Half-1 gathers are issued one group late
                so the early half-0 gathers only wait on the first AllGather."""
                if felem is None:
                    felem = fout
                g_of = plan.g_of
                tabs = (t1, t2)
                ng = len(plan.groups)
                lead = 8  # groups of half-0 gathers issued ahead of half-1
                seq = []
                for g in range(ng + lead):
                    if g < ng:
                        seq.append((g, 0))
                    if g >= lead:
                        seq.append((g - lead, 1))
                pos_of = {gh: i for i, gh in enumerate(seq)}
                gt = {}
                nissued = 0

                def issue_upto(idx):
                    nonlocal nissued
                    while nissued <= idx:
                        gi2, hh = seq[nissued]
                        if nissued == lead and post_issue is not None:
                            # trace the deferred collective as late as the
                            # first half-1 gather allows: the straight-line
                            # collective sem is coarse, so anything traced
                            # after it waits for it
                            post_issue()
                        nissued += 1
                        nid = plan.gather_nid[(gi2, hh)]
                        if nid == 0:
                            gt[(gi2, hh)] = None
                            continue
                        ncol = (nid + P - 1) // P
                        g_tile = gpool.tile(
                            [P, ncol, felem], hdt, tag="gath",
                            name=f"g{gi2}_{hh}",
                        )
                        i0 = plan.seg_idx16[(gi2, hh)]
                        nc.gpsimd.dma_gather(
                            g_tile[:],
                            tabs[hh][:, :],
                            idx_t[:, i0 : i0 + nid // 16],
                            nid,
                            nid,
                            felem,
                            single_packet=False,
                            queue_num=next(qctr) % 4,
                        )
                        gt[(gi2, hh)] = g_tile

                for gi, blocks in enumerate(plan.groups):
                    issue_upto(pos_of[(gi, 1)])
                    for b in blocks:
                        wb = min(P, npc - b * P)
                        pb = pscat.tile([P, fout], F32, name="pb")
                        nc.tensor.matmul(
                            pb[:],
                            sqrow_t[0:1, b * P : (b + 1) * P],
                            bias_t[:],
                            start=True,
                            stop=False,
                        )
                        pieces = []  # (h, col, p0, p1)
                        sels = {}
                        spans = {}  # h -> (first_col, ncols)
                        for h in (0, 1):
                            sz = int(plan.SZ[b, h])
                            if sz == 0:
                                continue
                            off = plan.seg_off[(b, h)]
                            c_lo = off // P
                            c_hi = (off + sz - 1) // P
                            spans[h] = (c_lo, c_hi - c_lo + 1)
                            for c in range(c_lo, c_hi + 1):
                                p0 = max(0, off - P * c)
                                p1 = min(P, off + sz - P * c)
                                pieces.append((h, c, p0, p1))
                        for h, (c_lo, nch) in spans.items():
                            assert nch <= 16, nch
                            colb = plan.seg_colbase[(g_of[b], h)]
                            sel = spool.tile(
                                [P, nch, P], hdt, tag="sel", name="sel"
                            )
                            nc.vector.tensor_tensor(
                                out=sel[:],
                                in0=slots_t[
                                    :, colb + c_lo : colb + c_lo + nch
                                ].to_broadcast([P, nch, P]),
                                in1=iotar_t[:, 0 : nch * P].rearrange(
                                    "p (a b) -> p a b", b=P
                                ),
                                op=mybir.AluOpType.is_equal,
                            )
                            sels[h] = (sel, c_lo)
                        for k, (h, c, p0, p1) in enumerate(pieces):
                            sel, c_lo = sels[h]
                            nc.tensor.matmul(
                                pb[:],
                                sel[p0:p1, c - c_lo, :],
                                gt[(gi, h)][p0:p1, c, 0:fout],
                                start=False,
                                stop=(k == len(pieces) - 1),
                            )
                        ob = stpool.tile([P, fout], F32, tag="ob", name="ob")
                        if is_last:
                            nc.scalar.activation(
                                ob[:wb, :],
                                pb[:wb, :],
                                AF.Copy,
                                scale=dinv_t[:wb, b : b + 1],
                            )
                            nc.sync.dma_start(
                                out=y_d[b * P : b * P + wb, :], in_=ob[:wb, :]
                            )
                        else:
                            # x1 = sigmoid(dinv*psum); immediately run this
                            # block's L1 GEMM into the h2 staging tile so the
                            # h2 half-tables publish as early as possible.
                            nc.scalar.activation(
                                ob[:],
                                pb[:],
                                AF.Sigmoid,
                                scale=dinv_t[:, b : b + 1],
                            )
                            pt = ptrans.tile([P, P], F32, name="pt")
                            nc.tensor.transpose(pt[:], ob[:], ident_t[:])
                            x1b = x1pool.tile([P, P], F32, name="x1b")
                            nc.vector.tensor_copy(x1b[:], pt[:])
                            hp2 = pgemm.tile([P, f2], F32, name="hp2")
                            nc.tensor.matmul(
                                hp2[:wb, :],
                                x1b[:, 0:wb],
                                w1_t[:],
                                start=True,
                                stop=True,
                            )
                            nc.scalar.activation(
                                h2sb[:wb, b, :],
                                hp2[:wb, :],
                                AF.Copy,
                                scale=dinv_t[:wb, b : b + 1],
                            )
                    if post_group is not None:
                        post_group(blocks)

            # ---- layer 0 ----
            gemm_layer0()

            def l0_post_group(blocks):
                # once all of table-1's source blocks have their L1 GEMM
                # staged, publish h2 table 1 (overlaps the rest of L0)
                if nsplit - 1 in blocks:
                    publish(h2sb, f2, h2_locA, h2_t1, 0, nsplit)

            scatter_layer(
                h1_t1, h1_t2, f1, b0_t, is_last=False, hdt=BF16,
                post_group=l0_post_group,
            )

            # ---- layer 1 ----  (h2 table-2 publishes after the first L1
            # half-0 gathers are traced: the straight-line collective sem is
            # coarse, so tracing it earlier would make them wait on it)
            scatter_layer(
                h2_t1, h2_t2, f2, b1_t, is_last=True, hdt=BF16, felem=P,
                post_issue=lambda: publish(h2sb, f2, h2_locB, h2_t2, nsplit, nblk),
            )

    nc.compile()
    return nc


def _make_in_maps(x, W0, b0, W1, b1, plan, per_core):
    npc = plan.npc
    x = np.asarray(x, dtype=np.float32)
    shared = dict(
        W0=np.asarray(W0, np.float32).reshape(W0.shape[0], -1),
        W1=np.asarray(W1, np.float32).reshape(W1.shape[0], -1),
        b0=np.asarray(b0, np.float32).reshape(1, -1),
        b1=np.asarray(b1, np.float32).reshape(1, -1),
        iota=np.tile(
            np.arange(P, dtype=np.float32)[None, :], (P, 1)
        ).astype(ml_dtypes.bfloat16),
        iotar=np.tile(
            np.arange(P, dtype=np.float32)[None, :], (P, 16)
        ).astype(ml_dtypes.bfloat16),
        ident=np.eye(P, dtype=np.float32),
    )
    in_maps = []
    for c in range(plan.n_cores):
        m = dict(shared)
        m["xT"] = np.ascontiguousarray(x[c * npc : (c + 1) * npc, :].T)
        m["idx16"] = per_core[c]["idx16"]
        m["widx"] = np.zeros((P, 8), np.int16)
        m["slots"] = per_core[c]["slots"].astype(ml_dtypes.bfloat16)
        m["degt"] = per_core[c]["degt"]
        m["degrow"] = per_core[c]["degrow"]
        in_maps.append(m)
    return in_maps


_CACHE = {}


def build(x, edges, W0, b0, W1, b1, n_nodes=N_NODES, n_cores=N_CORES,
          gb=GROUP_BLOCKS):
    """Returns (nc, in_maps, plan). Cached on the edge structure size."""
    plan, per_core = _build_metadata(edges, n_nodes, n_cores, gb)
    key = (n_nodes, n_cores, gb, tuple(plan.SZ.reshape(-1).tolist()))
    if key not in _CACHE:
        _CACHE[key] = _build_nc(plan, x.shape[1], W0.shape[1], W1.shape[1])
    nc = _CACHE[key]
    in_maps = _make_in_maps(x, W0, b0, W1, b1, plan, per_core)
    return nc, in_maps, plan


def kernel(x, edges, W0, b0, W1, b1):
    x = np.asarray(x)
    nc, in_maps, plan = build(x, edges, W0, b0, W1, b1)
    res = run_bass_kernel_spmd(nc, in_maps, list(range(plan.n_cores)))
    y = np.concatenate([r["y"] for r in res.results], axis=0)
    return y.astype(np.float32)



# revision 67
# speedup vs baseline: 1.5506x; 1.5506x over previous
"""Bass/Trainium2 SPMD kernel for a 2-layer GCN encoder.

Math (per reference):
    src/dst = edges + self-loops
    deg[v]  = #edges with dst==v (incl self-loop);  dinv = 1/sqrt(deg)
    layer(x, W, b): out[d] = dinv[d] * sum_{e: dst_e==d} dinv[src_e] * (x@W)[src_e] + b
    y = layer1(sigmoid(layer0(x, W0, b0)), W1, b1)

Distribution: nodes are sharded contiguously across 8 cores (6250 each).
Edges are owned by the destination core.  Each core:
  1. GEMM on its x rows, pre-scales rows by dinv (so the per-edge weight
     dinv[src]*dinv[dst] factorizes into a row pre-scale and an output
     post-scale), AllGathers the scaled features.
  2. For each 128-row destination block, gathers the source rows of its
     edges (dma_gather, int16 indices => the node table is split in two
     halves), builds one-hot scatter matrices on the vector engine
     (iota == slot), and scatter-adds via TensorE matmuls accumulating in
     PSUM.  Bias enters as a rank-1 matmul (sqrt(deg) x b), so the final
     PSUM->SBUF copy can apply the dinv post-scale (and sigmoid) in one
     ScalarE activation.
"""

import itertools
import math

import ml_dtypes
import numpy as np

import concourse.bacc as bacc
import concourse.bass as bass
import concourse.mybir as mybir
import concourse.tile as tile
from concourse.bass_utils import run_bass_kernel_spmd

P = 128
F32 = mybir.dt.float32
BF16 = mybir.dt.bfloat16
I16 = mybir.dt.int16

# Full-problem constants
N_NODES = 50000
N_CORES = 8
F0, F1, F2 = 128, 128, 64
GROUP_BLOCKS = 2  # dst blocks per dma_gather batch
# Per-(block,half) edge-segment alignment. Must stay 128: sub-128 matmul
# pieces with different base partitions back-to-back hard-crash the PE
# (verified on HW: K64@p0 directly followed by K64@p64 aborts the NEFF).
SEG_ALIGN = 128


def _round_up(x, m):
    return (x + m - 1) // m * m


class Plan:
    """Compile-time schedule, identical across cores (SPMD)."""

    def __init__(self, n_nodes, n_cores, gb):
        assert n_nodes % n_cores == 0
        self.n_nodes = n_nodes
        self.n_cores = n_cores
        self.npc = n_nodes // n_cores
        self.nblk = math.ceil(self.npc / P)
        # The feature table is AllGathered in two per-core row ranges
        # ([0,S1) and [S1,S1+S2) local rows, S2 padded to full blocks) so the
        # first half-table is available early and both halves stay under the
        # int16 gather-index limit.  S2 is made as large as int16 allows so
        # the first AllGather (after S1 GEMM blocks) fires as early as
        # possible.
        self.S2 = 32768 // n_cores // P * P
        self.S1 = self.nblk * P - self.S2
        assert 0 < self.S1 and n_cores * self.S1 <= 32768
        self.gb = gb
        self.groups = [
            list(range(i, min(i + gb, self.nblk))) for i in range(0, self.nblk, gb)
        ]
        self.g_of = {}
        for gi, blocks in enumerate(self.groups):
            for b in blocks:
                self.g_of[b] = gi
        # filled by finalize(): per-(blk, half) uniform padded sizes
        self.SZ = None  # [nblk, 2] int, multiples of SEG_ALIGN
        self.seg_off = {}  # (b, h) -> edge offset within its gather
        self.seg_idx16 = {}  # (g_idx, h) -> int16-column base of that gather
        self.seg_colbase = {}  # (g_idx, h) -> global chunk-column base
        self.gather_nid = {}  # (g_idx, h) -> num idxs
        self.ncols = 0
        self.tot16 = 0

    def finalize(self, sz):
        self.SZ = sz
        col = 0
        for gi, blocks in enumerate(self.groups):
            for h in (0, 1):
                off = 0
                for b in blocks:
                    self.seg_off[(b, h)] = off
                    off += int(self.SZ[b, h])
                self.gather_nid[(gi, h)] = off
                self.seg_colbase[(gi, h)] = col
                col += (off + P - 1) // P
        self.nid_max = _round_up(max(self.gather_nid.values()), P)
        i16 = 0
        for gi in range(len(self.groups)):
            for h in (0, 1):
                self.seg_idx16[(gi, h)] = i16
                i16 += self.gather_nid[(gi, h)] // 16
        self.ncols = col
        self.tot16 = i16


def _build_metadata(edges, n_nodes, n_cores, gb=GROUP_BLOCKS):
    """Host-side integer preprocessing: shard + sort edges, build gather
    indices / slot vectors / degree tables.  Returns (plan, per_core dict)."""
    plan = Plan(n_nodes, n_cores, gb)
    npc, nblk = plan.npc, plan.nblk
    S1, S2 = plan.S1, plan.S2

    loop = np.arange(n_nodes, dtype=np.int64)
    src = np.concatenate([np.asarray(edges[0], dtype=np.int64), loop])
    dst = np.concatenate([np.asarray(edges[1], dtype=np.int64), loop])
    deg = np.bincount(dst, minlength=n_nodes).astype(np.float32)

    owner = dst // npc
    ldst = dst % npc
    blk = ldst // P
    slot = (ldst % P).astype(np.float32)
    # gather-table position of each source node: half 0 = local rows [0,S1)
    # of every core (table1), half 1 = local rows [S1,S1+S2) (table2)
    sown = src // npc
    srow = src % npc
    half = (srow >= S1).astype(np.int64)
    tpos = np.where(half == 0, sown * S1 + srow, sown * S2 + (srow - S1))
    cell = ((owner * nblk) + blk) * 2 + half
    order = np.lexsort((src, cell))
    cell_s = cell[order]
    tpos_s = tpos[order]
    slot_s = slot[order]

    ncells = n_cores * nblk * 2
    counts = np.bincount(cell_s, minlength=ncells).reshape(n_cores, nblk, 2)
    starts = np.concatenate([[0], np.cumsum(counts.reshape(-1))])[:-1].reshape(
        n_cores, nblk, 2
    )
    sz = np.maximum(counts.max(axis=0), 0)
    sz = (np.ceil(sz / SEG_ALIGN).astype(np.int64)) * SEG_ALIGN  # [nblk, 2]
    plan.finalize(sz)

    ncols = plan.ncols
    tot16 = plan.tot16

    per_core = []
    for c in range(n_cores):
        idx16 = np.zeros((16, tot16), np.int16)
        slots_t = np.full((P, ncols), -1.0, np.float32)
        for gi, blocks in enumerate(plan.groups):
            for h in (0, 1):
                i16b = plan.seg_idx16[(gi, h)] * 16
                colb = plan.seg_colbase[(gi, h)] * P
                for b in blocks:
                    n = int(counts[c, b, h])
                    s0 = int(starts[c, b, h])
                    if n:
                        j = plan.seg_off[(b, h)] + np.arange(n)
                        seg_src = tpos_s[s0 : s0 + n].astype(np.int16)
                        ji = i16b + j
                        idx16[ji % 16, ji // 16] = seg_src
                        jc = colb + j
                        slots_t[jc % P, jc // P] = slot_s[s0 : s0 + n]
        deg_loc = np.ones(nblk * P, np.float32)
        deg_loc[:npc] = deg[c * npc : (c + 1) * npc]
        deg_t = deg_loc.reshape(nblk, P).T.copy()  # [P, nblk]
        per_core.append(
            dict(
                idx16=np.tile(idx16, (8, 1)),  # [128, tot16]
                slots=slots_t,
                degt=deg_t,
                degrow=deg_loc.reshape(1, -1).copy(),
            )
        )
    return plan, per_core


def _build_nc(plan, f0, f1, f2):
    """Build the SPMD bass program (same for every core)."""
    npc, nblk = plan.npc, plan.nblk
    S1, S2 = plan.S1, plan.S2
    ncores = plan.n_cores
    nsplit = S1 // P  # GEMM blocks staged into table 1
    nc = bacc.Bacc(
        "TRN2",
        target_bir_lowering=False,
        debug=False,
        num_devices=plan.n_cores,
        num_swdge_queues=4,
    )

    # I/O
    xT_d = nc.dram_tensor("xT", [f0, npc], F32, kind="ExternalInput")
    w0_d = nc.dram_tensor("W0", [f0, f1], F32, kind="ExternalInput")
    w1_d = nc.dram_tensor("W1", [f1, f2], F32, kind="ExternalInput")
    b0_d = nc.dram_tensor("b0", [1, f1], F32, kind="ExternalInput")
    b1_d = nc.dram_tensor("b1", [1, f2], F32, kind="ExternalInput")
    iota_d = nc.dram_tensor("iota", [P, P], BF16, kind="ExternalInput")
    iotar_d = nc.dram_tensor("iotar", [P, 16 * P], BF16, kind="ExternalInput")
    ident_d = nc.dram_tensor("ident", [P, P], F32, kind="ExternalInput")
    degt_d = nc.dram_tensor("degt", [P, nblk], F32, kind="ExternalInput")
    degrow_d = nc.dram_tensor("degrow", [1, nblk * P], F32, kind="ExternalInput")
    idx_d = nc.dram_tensor("idx16", [P, plan.tot16], I16, kind="ExternalInput")
    widx_d = nc.dram_tensor("widx", [P, 8], I16, kind="ExternalInput")
    slots_d = nc.dram_tensor("slots", [P, plan.ncols], BF16, kind="ExternalInput")
    y_d = nc.dram_tensor("y", [npc, f2], F32, kind="ExternalOutput")

    rg = [list(range(plan.n_cores))]
    AF = mybir.ActivationFunctionType

    with tile.TileContext(nc) as tc:
        with (
            tc.tile_pool(name="dram", bufs=1, space="DRAM") as dramp,
            tc.tile_pool(name="const", bufs=1) as constp,
            tc.tile_pool(name="gath", bufs=11) as gpool,
            tc.tile_pool(name="sel", bufs=5) as spool,
            tc.tile_pool(name="stage", bufs=4) as stpool,
            tc.tile_pool(name="x1p", bufs=4) as x1pool,
            tc.tile_pool(name="pgemm", bufs=2, space="PSUM") as pgemm,
            tc.tile_pool(name="pscat", bufs=3, space="PSUM") as pscat,
            tc.tile_pool(name="ptrans", bufs=1, space="PSUM") as ptrans,
        ):
            # Per-layer feature tables, AllGathered in two row ranges so the
            # first collective (and the half-0 gathers) fire early.  L1
            # tables are bf16 padded to 128 cols (gather elem must be a
            # multiple of 256B); cols f2:128 are never-read garbage.
            h1_locA = dramp.tile([S1, f1], BF16, name="h1_locA")
            h1_locB = dramp.tile([S2, f1], BF16, name="h1_locB")
            h1_t1 = dramp.tile(
                [ncores * S1, f1], BF16, addr_space="Shared", name="h1_t1"
            )
            h1_t2 = dramp.tile(
                [ncores * S2, f1], BF16, addr_space="Shared", name="h1_t2"
            )
            h2_locA = dramp.tile([S1, P], BF16, name="h2_locA")
            h2_locB = dramp.tile([S2, P], BF16, name="h2_locB")
            h2_t1 = dramp.tile(
                [ncores * S1, P], BF16, addr_space="Shared", name="h2_t1"
            )
            h2_t2 = dramp.tile(
                [ncores * S2, P], BF16, addr_space="Shared", name="h2_t2"
            )

            # ---- constants / metadata ----
            def load_const(name, dram, shape, dtype=F32):
                t = constp.tile(shape, dtype, name=name)
                nc.sync.dma_start(out=t[:], in_=dram[:])
                return t

            # ordered so the warm gathers + L0 GEMM -> AllGather chain start
            # ASAP; the big gather metadata loads overlap with it
            widx_t = load_const("widx_t", widx_d, [P, 8], I16)
            ident_t = load_const("ident_t", ident_d, [P, P])
            xT_t = load_const("xT_t", xT_d, [f0, npc])
            w0_t = load_const("w0_t", w0_d, [f0, f1])
            degt_t = load_const("degt_t", degt_d, [P, nblk])
            w1_t = load_const("w1_t", w1_d, [f1, f2])
            b0_t = load_const("b0_t", b0_d, [1, f1])
            b1_t = load_const("b1_t", b1_d, [1, f2])
            iota_t = load_const("iota_t", iota_d, [P, P], BF16)
            iotar_t = load_const("iotar_t", iotar_d, [P, 16 * P], BF16)
            degrow_t = load_const("degrow_t", degrow_d, [1, nblk * P])
            idx_t = load_const("idx_t", idx_d, [P, plan.tot16], I16)
            slots_t = load_const("slots_t", slots_d, [P, plan.ncols], BF16)

            # dinv = 1/sqrt(deg); sqdeg rows (flat, partition 0) for bias matmuls
            sq_t = constp.tile([P, nblk], F32, name="sq_t")
            nc.scalar.activation(sq_t[:], degt_t[:], AF.Sqrt)
            dinv_t = constp.tile([P, nblk], F32, name="dinv_t")
            nc.vector.reciprocal(dinv_t[:], sq_t[:])
            sqrow_t = constp.tile([1, nblk * P], F32, name="sqrow_t")
            nc.scalar.activation(sqrow_t[:], degrow_t[:], AF.Sqrt)

            # warm the Q7 dma_gather ucode on every SWDGE queue pair (each
            # pair pays its own ~29us icache fill; do it under the
            # GEMM+AllGather head instead)
            for q in range(4):
                warm_t = constp.tile([P, 1, 64], F32, name=f"warm_t{q}")
                nc.gpsimd.dma_gather(
                    warm_t[:],
                    ident_d[:, 0:64],
                    widx_t[:, 0:8],
                    128,
                    128,
                    64,
                    elem_step=P,
                    single_packet=False,
                    queue_num=q,
                )

            # SBUF staging for the local h rows of each layer: blocks land
            # here from the GEMMs, then two bulk DMAs + two AllGathers per
            # layer publish them as the gather tables.
            h1sb = constp.tile([P, nblk, f1], BF16, name="h1sb")
            h2sb = constp.tile([P, nblk, f2], BF16, name="h2sb")

            def publish(hsb, fstage, loc, tab, b_lo, b_hi):
                """DMA staged blocks [b_lo,b_hi) to local DRAM + AllGather.
                The DMA goes on the Scalar HWDGE queue so it does not queue
                behind the bulk constant loads on Sync."""
                nc.scalar.dma_start(
                    out=loc[:, 0:fstage].rearrange("(c p) f -> p c f", p=P),
                    in_=hsb[:, b_lo:b_hi, :],
                )
                nc.gpsimd.collective_compute(
                    "AllGather",
                    mybir.AluOpType.bypass,
                    replica_groups=rg,
                    ins=[loc[:, :].opt()],
                    outs=[tab[:, :].opt()],
                )

            def gemm_layer0():
                """h1sb[:, t] = dinv * (x @ W0); publish each half-table."""
                for t in range(nblk):
                    wt = min(P, npc - t * P)
                    hp = pgemm.tile([P, f1], F32, name="hp")
                    nc.tensor.matmul(
                        hp[:wt, :],
                        xT_t[:, t * P : t * P + wt],
                        w0_t[:],
                        start=True,
                        stop=True,
                    )
                    nc.scalar.activation(
                        h1sb[:wt, t, :],
                        hp[:wt, :],
                        AF.Copy,
                        scale=dinv_t[:wt, t : t + 1],
                    )
                    if t == nsplit - 1:
                        publish(h1sb, f1, h1_locA, h1_t1, 0, nsplit)
                publish(h1sb, f1, h1_locB, h1_t2, nsplit, nblk)

            qctr = itertools.count()

            def scatter_layer(
                t1, t2, fout, bias_t, is_last, hdt, felem=None,
                post_group=None, post_issue=None,
            ):
                """For every dst block: gather + one-hot matmul scatter-add.

                felem: gathered row width (table columns); the matmul only
                consumes the first `fout` of them.  Gathers rotate across the
                4 SWDGE queues so descriptor generation runs on all 4 Q7 core
                pairs concurrently.  Half-1 gathers are issued one group late
                so the early half-0 gathers only wait on the first AllGather."""
                if felem is None:
                    felem = fout
                g_of = plan.g_of
                tabs = (t1, t2)
                ng = len(plan.groups)
                lead = 4  # groups of half-0 gathers issued ahead of half-1
                seq = []
                for g in range(ng + lead):
                    if g < ng:
                        seq.append((g, 0))
                    if g >= lead:
                        seq.append((g - lead, 1))
                pos_of = {gh: i for i, gh in enumerate(seq)}
                gt = {}
                nissued = 0

                def issue_upto(idx):
                    nonlocal nissued
                    while nissued <= idx:
                        gi2, hh = seq[nissued]
                        if nissued == lead and post_issue is not None:
                            # trace the deferred collective as late as the
                            # first half-1 gather allows: the straight-line
                            # collective sem is coarse, so anything traced
                            # after it waits for it
                            post_issue()
                        nissued += 1
                        nid = plan.gather_nid[(gi2, hh)]
                        if nid == 0:
                            gt[(gi2, hh)] = None
                            continue
                        ncol = (nid + P - 1) // P
                        g_tile = gpool.tile(
                            [P, ncol, felem], hdt, tag="gath",
                            name=f"g{gi2}_{hh}",
                        )
                        i0 = plan.seg_idx16[(gi2, hh)]
                        nc.gpsimd.dma_gather(
                            g_tile[:],
                            tabs[hh][:, :],
                            idx_t[:, i0 : i0 + nid // 16],
                            nid,
                            nid,
                            felem,
                            single_packet=False,
                            queue_num=next(qctr) % 4,
                        )
                        gt[(gi2, hh)] = g_tile

                for gi, blocks in enumerate(plan.groups):
                    issue_upto(pos_of[(gi, 1)])
                    for b in blocks:
                        wb = min(P, npc - b * P)
                        pb = pscat.tile([P, fout], F32, name="pb")
                        nc.tensor.matmul(
                            pb[:],
                            sqrow_t[0:1, b * P : (b + 1) * P],
                            bias_t[:],
                            start=True,
                            stop=False,
                        )
                        pieces = []  # (h, col, p0, p1)
                        sels = {}
                        spans = {}  # h -> (first_col, ncols)
                        for h in (0, 1):
                            sz = int(plan.SZ[b, h])
                            if sz == 0:
                                continue
                            off = plan.seg_off[(b, h)]
                            c_lo = off // P
                            c_hi = (off + sz - 1) // P
                            spans[h] = (c_lo, c_hi - c_lo + 1)
                            for c in range(c_lo, c_hi + 1):
                                p0 = max(0, off - P * c)
                                p1 = min(P, off + sz - P * c)
                                pieces.append((h, c, p0, p1))
                        for h, (c_lo, nch) in spans.items():
                            assert nch <= 16, nch
                            colb = plan.seg_colbase[(g_of[b], h)]
                            sel = spool.tile(
                                [P, nch, P], hdt, tag="sel", name="sel"
                            )
                            nc.vector.tensor_tensor(
                                out=sel[:],
                                in0=slots_t[
                                    :, colb + c_lo : colb + c_lo + nch
                                ].to_broadcast([P, nch, P]),
                                in1=iotar_t[:, 0 : nch * P].rearrange(
                                    "p (a b) -> p a b", b=P
                                ),
                                op=mybir.AluOpType.is_equal,
                            )
                            sels[h] = (sel, c_lo)
                        for k, (h, c, p0, p1) in enumerate(pieces):
                            sel, c_lo = sels[h]
                            nc.tensor.matmul(
                                pb[:],
                                sel[p0:p1, c - c_lo, :],
                                gt[(gi, h)][p0:p1, c, 0:fout],
                                start=False,
                                stop=(k == len(pieces) - 1),
                            )
                        ob = stpool.tile([P, fout], F32, tag="ob", name="ob")
                        if is_last:
                            nc.scalar.activation(
                                ob[:wb, :],
                                pb[:wb, :],
                                AF.Copy,
                                scale=dinv_t[:wb, b : b + 1],
                            )
                            nc.sync.dma_start(
                                out=y_d[b * P : b * P + wb, :], in_=ob[:wb, :]
                            )
                        else:
                            # x1 = sigmoid(dinv*psum); immediately run this
                            # block's L1 GEMM into the h2 staging tile so the
                            # h2 half-tables publish as early as possible.
                            nc.scalar.activation(
                                ob[:],
                                pb[:],
                                AF.Sigmoid,
                                scale=dinv_t[:, b : b + 1],
                            )
                            pt = ptrans.tile([P, P], F32, name="pt")
                            nc.tensor.transpose(pt[:], ob[:], ident_t[:])
                            x1b = x1pool.tile([P, P], F32, name="x1b")
                            nc.vector.tensor_copy(x1b[:], pt[:])
                            hp2 = pgemm.tile([P, f2], F32, name="hp2")
                            nc.tensor.matmul(
                                hp2[:wb, :],
                                x1b[:, 0:wb],
                                w1_t[:],
                                start=True,
                                stop=True,
                            )
                            nc.scalar.activation(
                                h2sb[:wb, b, :],
                                hp2[:wb, :],
                                AF.Copy,
                                scale=dinv_t[:wb, b : b + 1],
                            )
                    if post_group is not None:
                        post_group(blocks)

            # ---- layer 0 ----
            gemm_layer0()

            def l0_post_group(blocks):
                # once all of table-1's source blocks have their L1 GEMM
                # staged, publish h2 table 1 (overlaps the rest of L0)
                if nsplit - 1 in blocks:
                    publish(h2sb, f2, h2_locA, h2_t1, 0, nsplit)

            scatter_layer(
                h1_t1, h1_t2, f1, b0_t, is_last=False, hdt=BF16,
                post_group=l0_post_group,
            )

            # ---- layer 1 ----  (h2 table-2 publishes after the first L1
            # half-0 gathers are traced: the straight-line collective sem is
            # coarse, so tracing it earlier would make them wait on it)
            scatter_layer(
                h2_t1, h2_t2, f2, b1_t, is_last=True, hdt=BF16, felem=P,
                post_issue=lambda: publish(h2sb, f2, h2_locB, h2_t2, nsplit, nblk),
            )

    nc.compile()
    return nc


def _make_in_maps(x, W0, b0, W1, b1, plan, per_core):
    npc = plan.npc
    x = np.asarray(x, dtype=np.float32)
    shared = dict(
        W0=np.asarray(W0, np.float32).reshape(W0.shape[0], -1),
        W1=np.asarray(W1, np.float32).reshape(W1.shape[0], -1),
        b0=np.asarray(b0, np.float32).reshape(1, -1),
        b1=np.asarray(b1, np.float32).reshape(1, -1),
        iota=np.tile(
            np.arange(P, dtype=np.float32)[None, :], (P, 1)
        ).astype(ml_dtypes.bfloat16),
        iotar=np.tile(
            np.arange(P, dtype=np.float32)[None, :], (P, 16)
        ).astype(ml_dtypes.bfloat16),
        ident=np.eye(P, dtype=np.float32),
    )
    in_maps = []
    for c in range(plan.n_cores):
        m = dict(shared)
        m["xT"] = np.ascontiguousarray(x[c * npc : (c + 1) * npc, :].T)
        m["idx16"] = per_core[c]["idx16"]
        m["widx"] = np.zeros((P, 8), np.int16)
        m["slots"] = per_core[c]["slots"].astype(ml_dtypes.bfloat16)
        m["degt"] = per_core[c]["degt"]
        m["degrow"] = per_core[c]["degrow"]
        in_maps.append(m)
    return in_maps


_CACHE = {}


def build(x, edges, W0, b0, W1, b1, n_nodes=N_NODES, n_cores=N_CORES,
          gb=GROUP_BLOCKS):
    """Returns (nc, in_maps, plan). Cached on the edge structure size."""
    plan, per_core = _build_metadata(edges, n_nodes, n_cores, gb)
    key = (n_nodes, n_cores, gb, tuple(plan.SZ.reshape(-1).tolist()))
    if key not in _CACHE:
        _CACHE[key] = _build_nc(plan, x.shape[1], W0.shape[1], W1.shape[1])
    nc = _CACHE[key]
    in_maps = _make_in_maps(x, W0, b0, W1, b1, plan, per_core)
    return nc, in_maps, plan


def kernel(x, edges, W0, b0, W1, b1):
    x = np.asarray(x)
    nc, in_maps, plan = build(x, edges, W0, b0, W1, b1)
    res = run_bass_kernel_spmd(nc, in_maps, list(range(plan.n_cores)))
    y = np.concatenate([r["y"] for r in res.results], axis=0)
    return y.astype(np.float32)



# revision 70
# speedup vs baseline: 1.5639x; 1.0085x over previous
"""Bass/Trainium2 SPMD kernel for a 2-layer GCN encoder.

Math (per reference):
    src/dst = edges + self-loops
    deg[v]  = #edges with dst==v (incl self-loop);  dinv = 1/sqrt(deg)
    layer(x, W, b): out[d] = dinv[d] * sum_{e: dst_e==d} dinv[src_e] * (x@W)[src_e] + b
    y = layer1(sigmoid(layer0(x, W0, b0)), W1, b1)

Distribution: nodes are sharded contiguously across 8 cores (6250 each).
Edges are owned by the destination core.  Each core:
  1. GEMM on its x rows, pre-scales rows by dinv (so the per-edge weight
     dinv[src]*dinv[dst] factorizes into a row pre-scale and an output
     post-scale), AllGathers the scaled features.
  2. For each 128-row destination block, gathers the source rows of its
     edges (dma_gather, int16 indices => the node table is split in two
     halves), builds one-hot scatter matrices on the vector engine
     (iota == slot), and scatter-adds via TensorE matmuls accumulating in
     PSUM.  Bias enters as a rank-1 matmul (sqrt(deg) x b), so the final
     PSUM->SBUF copy can apply the dinv post-scale (and sigmoid) in one
     ScalarE activation.
"""

import itertools
import math

import ml_dtypes
import numpy as np

import concourse.bacc as bacc
import concourse.bass as bass
import concourse.mybir as mybir
import concourse.tile as tile
from concourse.bass_utils import run_bass_kernel_spmd

P = 128
F32 = mybir.dt.float32
BF16 = mybir.dt.bfloat16
I16 = mybir.dt.int16

# Full-problem constants
N_NODES = 50000
N_CORES = 8
F0, F1, F2 = 128, 128, 64
GROUP_BLOCKS = 2  # dst blocks per dma_gather batch
# Per-(block,half) edge-segment alignment. Must stay 128: sub-128 matmul
# pieces with different base partitions back-to-back hard-crash the PE
# (verified on HW: K64@p0 directly followed by K64@p64 aborts the NEFF).
SEG_ALIGN = 128


def _round_up(x, m):
    return (x + m - 1) // m * m


class Plan:
    """Compile-time schedule, identical across cores (SPMD)."""

    def __init__(self, n_nodes, n_cores, gb):
        assert n_nodes % n_cores == 0
        self.n_nodes = n_nodes
        self.n_cores = n_cores
        self.npc = n_nodes // n_cores
        self.nblk = math.ceil(self.npc / P)
        # The feature table is AllGathered in two per-core row ranges
        # ([0,S1) and [S1,S1+S2) local rows, S2 padded to full blocks) so the
        # first half-table is available early and both halves stay under the
        # int16 gather-index limit.  S2 is made as large as int16 allows so
        # the first AllGather (after S1 GEMM blocks) fires as early as
        # possible.
        self.S2 = 32768 // n_cores // P * P
        self.S1 = self.nblk * P - self.S2
        assert 0 < self.S1 and n_cores * self.S1 <= 32768
        self.gb = gb
        self.groups = [
            list(range(i, min(i + gb, self.nblk))) for i in range(0, self.nblk, gb)
        ]
        self.g_of = {}
        for gi, blocks in enumerate(self.groups):
            for b in blocks:
                self.g_of[b] = gi
        # filled by finalize(): per-(blk, half) uniform padded sizes
        self.SZ = None  # [nblk, 2] int, multiples of SEG_ALIGN
        self.seg_off = {}  # (b, h) -> edge offset within its gather
        self.seg_idx16 = {}  # (g_idx, h) -> int16-column base of that gather
        self.seg_colbase = {}  # (g_idx, h) -> global chunk-column base
        self.gather_nid = {}  # (g_idx, h) -> num idxs
        self.ncols = 0
        self.tot16 = 0

    def finalize(self, sz):
        self.SZ = sz
        col = 0
        for gi, blocks in enumerate(self.groups):
            for h in (0, 1):
                off = 0
                for b in blocks:
                    self.seg_off[(b, h)] = off
                    off += int(self.SZ[b, h])
                self.gather_nid[(gi, h)] = off
                self.seg_colbase[(gi, h)] = col
                col += (off + P - 1) // P
        self.nid_max = _round_up(max(self.gather_nid.values()), P)
        i16 = 0
        for gi in range(len(self.groups)):
            for h in (0, 1):
                self.seg_idx16[(gi, h)] = i16
                i16 += self.gather_nid[(gi, h)] // 16
        self.ncols = col
        self.tot16 = i16


def _build_metadata(edges, n_nodes, n_cores, gb=GROUP_BLOCKS):
    """Host-side integer preprocessing: shard + sort edges, build gather
    indices / slot vectors / degree tables.  Returns (plan, per_core dict)."""
    plan = Plan(n_nodes, n_cores, gb)
    npc, nblk = plan.npc, plan.nblk
    S1, S2 = plan.S1, plan.S2

    loop = np.arange(n_nodes, dtype=np.int64)
    src = np.concatenate([np.asarray(edges[0], dtype=np.int64), loop])
    dst = np.concatenate([np.asarray(edges[1], dtype=np.int64), loop])
    deg = np.bincount(dst, minlength=n_nodes).astype(np.float32)

    owner = dst // npc
    ldst = dst % npc
    blk = ldst // P
    slot = (ldst % P).astype(np.float32)
    # gather-table position of each source node: half 0 = local rows [0,S1)
    # of every core (table1), half 1 = local rows [S1,S1+S2) (table2)
    sown = src // npc
    srow = src % npc
    half = (srow >= S1).astype(np.int64)
    tpos = np.where(half == 0, sown * S1 + srow, sown * S2 + (srow - S1))
    cell = ((owner * nblk) + blk) * 2 + half
    order = np.lexsort((src, cell))
    cell_s = cell[order]
    tpos_s = tpos[order]
    slot_s = slot[order]

    ncells = n_cores * nblk * 2
    counts = np.bincount(cell_s, minlength=ncells).reshape(n_cores, nblk, 2)
    starts = np.concatenate([[0], np.cumsum(counts.reshape(-1))])[:-1].reshape(
        n_cores, nblk, 2
    )
    sz = np.maximum(counts.max(axis=0), 0)
    sz = (np.ceil(sz / SEG_ALIGN).astype(np.int64)) * SEG_ALIGN  # [nblk, 2]
    plan.finalize(sz)

    ncols = plan.ncols
    tot16 = plan.tot16

    per_core = []
    for c in range(n_cores):
        idx16 = np.zeros((16, tot16), np.int16)
        slots_t = np.full((P, ncols), -1.0, np.float32)
        for gi, blocks in enumerate(plan.groups):
            for h in (0, 1):
                i16b = plan.seg_idx16[(gi, h)] * 16
                colb = plan.seg_colbase[(gi, h)] * P
                for b in blocks:
                    n = int(counts[c, b, h])
                    s0 = int(starts[c, b, h])
                    if n:
                        j = plan.seg_off[(b, h)] + np.arange(n)
                        seg_src = tpos_s[s0 : s0 + n].astype(np.int16)
                        ji = i16b + j
                        idx16[ji % 16, ji // 16] = seg_src
                        jc = colb + j
                        slots_t[jc % P, jc // P] = slot_s[s0 : s0 + n]
        deg_loc = np.ones(nblk * P, np.float32)
        deg_loc[:npc] = deg[c * npc : (c + 1) * npc]
        deg_t = deg_loc.reshape(nblk, P).T.copy()  # [P, nblk]
        per_core.append(
            dict(
                idx16=np.tile(idx16, (8, 1)),  # [128, tot16]
                slots=slots_t,
                degt=deg_t,
                degrow=deg_loc.reshape(1, -1).copy(),
            )
        )
    return plan, per_core


def _build_nc(plan, f0, f1, f2):
    """Build the SPMD bass program (same for every core)."""
    npc, nblk = plan.npc, plan.nblk
    S1, S2 = plan.S1, plan.S2
    ncores = plan.n_cores
    nsplit = S1 // P  # GEMM blocks staged into table 1
    nc = bacc.Bacc(
        "TRN2",
        target_bir_lowering=False,
        debug=False,
        num_devices=plan.n_cores,
        num_swdge_queues=4,
    )

    # I/O
    xT_d = nc.dram_tensor("xT", [f0, npc], F32, kind="ExternalInput")
    w0_d = nc.dram_tensor("W0", [f0, f1], F32, kind="ExternalInput")
    w1_d = nc.dram_tensor("W1", [f1, f2], F32, kind="ExternalInput")
    b0_d = nc.dram_tensor("b0", [1, f1], F32, kind="ExternalInput")
    b1_d = nc.dram_tensor("b1", [1, f2], F32, kind="ExternalInput")
    iota_d = nc.dram_tensor("iota", [P, P], BF16, kind="ExternalInput")
    iotar_d = nc.dram_tensor("iotar", [P, 16 * P], BF16, kind="ExternalInput")
    ident_d = nc.dram_tensor("ident", [P, P], F32, kind="ExternalInput")
    degt_d = nc.dram_tensor("degt", [P, nblk], F32, kind="ExternalInput")
    degrow_d = nc.dram_tensor("degrow", [1, nblk * P], F32, kind="ExternalInput")
    idx_d = nc.dram_tensor("idx16", [P, plan.tot16], I16, kind="ExternalInput")
    widx_d = nc.dram_tensor("widx", [P, 8], I16, kind="ExternalInput")
    slots_d = nc.dram_tensor("slots", [P, plan.ncols], BF16, kind="ExternalInput")
    y_d = nc.dram_tensor("y", [npc, f2], F32, kind="ExternalOutput")

    rg = [list(range(plan.n_cores))]
    AF = mybir.ActivationFunctionType

    with tile.TileContext(nc) as tc:
        with (
            tc.tile_pool(name="dram", bufs=1, space="DRAM") as dramp,
            tc.tile_pool(name="const", bufs=1) as constp,
            tc.tile_pool(name="gath", bufs=10) as gpool,
            tc.tile_pool(name="sel", bufs=6) as spool,
            tc.tile_pool(name="stage", bufs=4) as stpool,
            tc.tile_pool(name="x1p", bufs=4) as x1pool,
            tc.tile_pool(name="pgemm", bufs=2, space="PSUM") as pgemm,
            tc.tile_pool(name="pscat", bufs=3, space="PSUM") as pscat,
            tc.tile_pool(name="ptrans", bufs=1, space="PSUM") as ptrans,
        ):
            # Per-layer feature tables, AllGathered in two row ranges so the
            # first collective (and the half-0 gathers) fire early.  L1
            # tables are bf16 padded to 128 cols (gather elem must be a
            # multiple of 256B); cols f2:128 are never-read garbage.
            h1_locA = dramp.tile([S1, f1], BF16, name="h1_locA")
            h1_locB = dramp.tile([S2, f1], BF16, name="h1_locB")
            h1_t1 = dramp.tile(
                [ncores * S1, f1], BF16, addr_space="Shared", name="h1_t1"
            )
            h1_t2 = dramp.tile(
                [ncores * S2, f1], BF16, addr_space="Shared", name="h1_t2"
            )
            h2_locA = dramp.tile([S1, P], BF16, name="h2_locA")
            h2_locB = dramp.tile([S2, P], BF16, name="h2_locB")
            h2_t1 = dramp.tile(
                [ncores * S1, P], BF16, addr_space="Shared", name="h2_t1"
            )
            h2_t2 = dramp.tile(
                [ncores * S2, P], BF16, addr_space="Shared", name="h2_t2"
            )

            # ---- constants / metadata ----
            def load_const(name, dram, shape, dtype=F32):
                t = constp.tile(shape, dtype, name=name)
                nc.sync.dma_start(out=t[:], in_=dram[:])
                return t

            # ordered so the L0 GEMM -> AllGather chain starts ASAP; the big
            # gather metadata loads overlap with it
            xT_t = load_const("xT_t", xT_d, [f0, npc])
            w0_t = load_const("w0_t", w0_d, [f0, f1])
            degt_t = load_const("degt_t", degt_d, [P, nblk])
            w1_t = load_const("w1_t", w1_d, [f1, f2])
            b0_t = load_const("b0_t", b0_d, [1, f1])
            b1_t = load_const("b1_t", b1_d, [1, f2])
            iota_t = load_const("iota_t", iota_d, [P, P], BF16)
            iotar_t = load_const("iotar_t", iotar_d, [P, 16 * P], BF16)
            ident_t = load_const("ident_t", ident_d, [P, P])
            degrow_t = load_const("degrow_t", degrow_d, [1, nblk * P])
            widx_t = load_const("widx_t", widx_d, [P, 8], I16)
            idx_t = load_const("idx_t", idx_d, [P, plan.tot16], I16)
            slots_t = load_const("slots_t", slots_d, [P, plan.ncols], BF16)

            # dinv = 1/sqrt(deg); sqdeg rows (flat, partition 0) for bias matmuls
            sq_t = constp.tile([P, nblk], F32, name="sq_t")
            nc.scalar.activation(sq_t[:], degt_t[:], AF.Sqrt)
            dinv_t = constp.tile([P, nblk], F32, name="dinv_t")
            nc.vector.reciprocal(dinv_t[:], sq_t[:])
            sqrow_t = constp.tile([1, nblk * P], F32, name="sqrow_t")
            nc.scalar.activation(sqrow_t[:], degrow_t[:], AF.Sqrt)

            # warm the Q7 dma_gather ucode on every SWDGE queue pair (each
            # pair pays its own ~29us icache fill; do it under the
            # GEMM+AllGather head instead)
            for q in range(4):
                warm_t = constp.tile([P, 1, 64], F32, name=f"warm_t{q}")
                nc.gpsimd.dma_gather(
                    warm_t[:],
                    ident_d[:, 0:64],
                    widx_t[:, 0:8],
                    128,
                    128,
                    64,
                    elem_step=P,
                    single_packet=False,
                    queue_num=q,
                )

            # SBUF staging for the local h rows of each layer: blocks land
            # here from the GEMMs, then two bulk DMAs + two AllGathers per
            # layer publish them as the gather tables.
            h1sb = constp.tile([P, nblk, f1], BF16, name="h1sb")
            h2sb = constp.tile([P, nblk, f2], BF16, name="h2sb")

            def publish(hsb, fstage, loc, tab, b_lo, b_hi):
                """DMA staged blocks [b_lo,b_hi) to local DRAM + AllGather."""
                nc.sync.dma_start(
                    out=loc[:, 0:fstage].rearrange("(c p) f -> p c f", p=P),
                    in_=hsb[:, b_lo:b_hi, :],
                )
                nc.gpsimd.collective_compute(
                    "AllGather",
                    mybir.AluOpType.bypass,
                    replica_groups=rg,
                    ins=[loc[:, :].opt()],
                    outs=[tab[:, :].opt()],
                )

            def gemm_layer0():
                """h1sb[:, t] = dinv * (x @ W0); publish each half-table."""
                for t in range(nblk):
                    wt = min(P, npc - t * P)
                    hp = pgemm.tile([P, f1], F32, name="hp")
                    nc.tensor.matmul(
                        hp[:wt, :],
                        xT_t[:, t * P : t * P + wt],
                        w0_t[:],
                        start=True,
                        stop=True,
                    )
                    nc.scalar.activation(
                        h1sb[:wt, t, :],
                        hp[:wt, :],
                        AF.Copy,
                        scale=dinv_t[:wt, t : t + 1],
                    )
                    if t == nsplit - 1:
                        publish(h1sb, f1, h1_locA, h1_t1, 0, nsplit)
                publish(h1sb, f1, h1_locB, h1_t2, nsplit, nblk)

            qctr = itertools.count()

            def scatter_layer(
                t1, t2, fout, bias_t, is_last, hdt, felem=None,
                post_group=None, post_issue=None,
            ):
                """For every dst block: gather + one-hot matmul scatter-add.

                felem: gathered row width (table columns); the matmul only
                consumes the first `fout` of them.  Gathers rotate across the
                4 SWDGE queues so descriptor generation runs on all 4 Q7 core
                pairs concurrently.  Half-1 gathers are issued one group late
                so the early half-0 gathers only wait on the first AllGather."""
                if felem is None:
                    felem = fout
                g_of = plan.g_of
                tabs = (t1, t2)
                ng = len(plan.groups)
                lead = 4  # groups of half-0 gathers issued ahead of half-1
                seq = []
                for g in range(ng + lead):
                    if g < ng:
                        seq.append((g, 0))
                    if g >= lead:
                        seq.append((g - lead, 1))
                pos_of = {gh: i for i, gh in enumerate(seq)}
                gt = {}
                nissued = 0

                def issue_upto(idx):
                    nonlocal nissued
                    while nissued <= idx:
                        gi2, hh = seq[nissued]
                        if nissued == lead and post_issue is not None:
                            # trace the deferred collective as late as the
                            # first half-1 gather allows: the straight-line
                            # collective sem is coarse, so anything traced
                            # after it waits for it
                            post_issue()
                        nissued += 1
                        nid = plan.gather_nid[(gi2, hh)]
                        if nid == 0:
                            gt[(gi2, hh)] = None
                            continue
                        ncol = (nid + P - 1) // P
                        g_tile = gpool.tile(
                            [P, ncol, felem], hdt, tag="gath",
                            name=f"g{gi2}_{hh}",
                        )
                        i0 = plan.seg_idx16[(gi2, hh)]
                        nc.gpsimd.dma_gather(
                            g_tile[:],
                            tabs[hh][:, :],
                            idx_t[:, i0 : i0 + nid // 16],
                            nid,
                            nid,
                            felem,
                            single_packet=False,
                            queue_num=next(qctr) % 4,
                        )
                        gt[(gi2, hh)] = g_tile

                for gi, blocks in enumerate(plan.groups):
                    issue_upto(pos_of[(gi, 1)])
                    for b in blocks:
                        wb = min(P, npc - b * P)
                        pb = pscat.tile([P, fout], F32, name="pb")
                        nc.tensor.matmul(
                            pb[:],
                            sqrow_t[0:1, b * P : (b + 1) * P],
                            bias_t[:],
                            start=True,
                            stop=False,
                        )
                        pieces = []  # (h, col, p0, p1)
                        sels = {}
                        spans = {}  # h -> (first_col, ncols)
                        for h in (0, 1):
                            sz = int(plan.SZ[b, h])
                            if sz == 0:
                                continue
                            off = plan.seg_off[(b, h)]
                            c_lo = off // P
                            c_hi = (off + sz - 1) // P
                            spans[h] = (c_lo, c_hi - c_lo + 1)
                            for c in range(c_lo, c_hi + 1):
                                p0 = max(0, off - P * c)
                                p1 = min(P, off + sz - P * c)
                                pieces.append((h, c, p0, p1))
                        for h, (c_lo, nch) in spans.items():
                            assert nch <= 16, nch
                            colb = plan.seg_colbase[(g_of[b], h)]
                            sel = spool.tile(
                                [P, nch, P], hdt, tag="sel", name="sel"
                            )
                            nc.vector.tensor_tensor(
                                out=sel[:],
                                in0=slots_t[
                                    :, colb + c_lo : colb + c_lo + nch
                                ].to_broadcast([P, nch, P]),
                                in1=iotar_t[:, 0 : nch * P].rearrange(
                                    "p (a b) -> p a b", b=P
                                ),
                                op=mybir.AluOpType.is_equal,
                            )
                            sels[h] = (sel, c_lo)
                        for k, (h, c, p0, p1) in enumerate(pieces):
                            sel, c_lo = sels[h]
                            nc.tensor.matmul(
                                pb[:],
                                sel[p0:p1, c - c_lo, :],
                                gt[(gi, h)][p0:p1, c, 0:fout],
                                start=False,
                                stop=(k == len(pieces) - 1),
                            )
                        ob = stpool.tile([P, fout], F32, tag="ob", name="ob")
                        if is_last:
                            nc.scalar.activation(
                                ob[:wb, :],
                                pb[:wb, :],
                                AF.Copy,
                                scale=dinv_t[:wb, b : b + 1],
                            )
                            nc.sync.dma_start(
                                out=y_d[b * P : b * P + wb, :], in_=ob[:wb, :]
                            )
                        else:
                            # x1 = sigmoid(dinv*psum); immediately run this
                            # block's L1 GEMM into the h2 staging tile so the
                            # h2 half-tables publish as early as possible.
                            nc.scalar.activation(
                                ob[:],
                                pb[:],
                                AF.Sigmoid,
                                scale=dinv_t[:, b : b + 1],
                            )
                            pt = ptrans.tile([P, P], F32, name="pt")
                            nc.tensor.transpose(pt[:], ob[:], ident_t[:])
                            x1b = x1pool.tile([P, P], F32, name="x1b")
                            nc.vector.tensor_copy(x1b[:], pt[:])
                            hp2 = pgemm.tile([P, f2], F32, name="hp2")
                            nc.tensor.matmul(
                                hp2[:wb, :],
                                x1b[:, 0:wb],
                                w1_t[:],
                                start=True,
                                stop=True,
                            )
                            nc.scalar.activation(
                                h2sb[:wb, b, :],
                                hp2[:wb, :],
                                AF.Copy,
                                scale=dinv_t[:wb, b : b + 1],
                            )
                    if post_group is not None:
                        post_group(blocks)

            # ---- layer 0 ----
            gemm_layer0()

            def l0_post_group(blocks):
                # once all of table-1's source blocks have their L1 GEMM
                # staged, publish h2 table 1 (overlaps the rest of L0)
                if nsplit - 1 in blocks:
                    publish(h2sb, f2, h2_locA, h2_t1, 0, nsplit)

            scatter_layer(
                h1_t1, h1_t2, f1, b0_t, is_last=False, hdt=BF16,
                post_group=l0_post_group,
            )

            # ---- layer 1 ----  (h2 table-2 publishes after the first L1
            # half-0 gathers are traced: the straight-line collective sem is
            # coarse, so tracing it earlier would make them wait on it)
            scatter_layer(
                h2_t1, h2_t2, f2, b1_t, is_last=True, hdt=BF16, felem=P,
                post_issue=lambda: publish(h2sb, f2, h2_locB, h2_t2, nsplit, nblk),
            )

    nc.compile()
    return nc


def _make_in_maps(x, W0, b0, W1, b1, plan, per_core):
    npc = plan.npc
    x = np.asarray(x, dtype=np.float32)
    shared = dict(
        W0=np.asarray(W0, np.float32).reshape(W0.shape[0], -1),
        W1=np.asarray(W1, np.float32).reshape(W1.shape[0], -1),
        b0=np.asarray(b0, np.float32).reshape(1, -1),
        b1=np.asarray(b1, np.float32).reshape(1, -1),
        iota=np.tile(
            np.arange(P, dtype=np.float32)[None, :], (P, 1)
        ).astype(ml_dtypes.bfloat16),
        iotar=np.tile(
            np.arange(P, dtype=np.float32)[None, :], (P, 16)
        ).astype(ml_dtypes.bfloat16),
        ident=np.eye(P, dtype=np.float32),
    )
    in_maps = []
    for c in range(plan.n_cores):
        m = dict(shared)
        m["xT"] = np.ascontiguousarray(x[c * npc : (c + 1) * npc, :].T)
        m["idx16"] = per_core[c]["idx16"]
        m["widx"] = np.zeros((P, 8), np.int16)
        m["slots"] = per_core[c]["slots"].astype(ml_dtypes.bfloat16)
        m["degt"] = per_core[c]["degt"]
        m["degrow"] = per_core[c]["degrow"]
        in_maps.append(m)
    return in_maps


_CACHE = {}


def build(x, edges, W0, b0, W1, b1, n_nodes=N_NODES, n_cores=N_CORES,
          gb=GROUP_BLOCKS):
    """Returns (nc, in_maps, plan). Cached on the edge structure size."""
    plan, per_core = _build_metadata(edges, n_nodes, n_cores, gb)
    key = (n_nodes, n_cores, gb, tuple(plan.SZ.reshape(-1).tolist()))
    if key not in _CACHE:
        _CACHE[key] = _build_nc(plan, x.shape[1], W0.shape[1], W1.shape[1])
    nc = _CACHE[key]
    in_maps = _make_in_maps(x, W0, b0, W1, b1, plan, per_core)
    return nc, in_maps, plan


def kernel(x, edges, W0, b0, W1, b1):
    x = np.asarray(x)
    nc, in_maps, plan = build(x, edges, W0, b0, W1, b1)
    res = run_bass_kernel_spmd(nc, in_maps, list(range(plan.n_cores)))
    y = np.concatenate([r["y"] for r in res.results], axis=0)
    return y.astype(np.float32)

